# revision 6
# baseline (speedup 1.0000x reference)
"""MoE expert-group kernel — bf16, slot-capacity packing, batched output DMA.

Measured HW facts this design is built on (see memory/trn2-moe-kernel-findings):
  - With >2 active cores the PE clock drops to ~2.0GHz (chip DVFS), so the
    8-core floor is ~1.2x the 2.4GHz roofline — PE cycles are the budget.
  - Slot packing: experts sorted by token count; the 8 largest go to slot 0
    (cap_hi), the 8 smallest to slot 1 (cap_lo <= cap_hi). Every core runs the
    same program shapes but on different experts; saves ~5% PE work vs padding
    all 16 experts to the global max.
  - Drains: L1 relu+bias on ACT (only function it ever runs), L2 bias on DVE,
    both amortized by PSUM bufs=4 (all 8 banks).
  - yt output batched: one DMA per (expert, tile) instead of per m-chunk
    (48 -> ~5 issues/iter) to cut 8-core DMA-queue interference.
"""

import contextlib
import os
import sys

import numpy as np

sys.path.insert(0, "/opt/trn_rl_repo")

N_TOKENS = 8192
D_MODEL = 1024
D_HIDDEN = 2048
N_EXPERTS = 16
TOP_K = 2
N_CORES = 8
EPC = N_EXPERTS // N_CORES
KC1 = D_MODEL // 128
MC1 = D_HIDDEN // 128
KC2 = D_HIDDEN // 128
MC2 = D_MODEL // 128


def _split_tiles(cap):
    """Moving-dim tiles <=512 (PSUM bank limit), multiples of 128."""
    if cap % 512 == 0:
        return [512] * (cap // 512)
    if cap % 384 == 0:
        return [384] * (cap // 384)
    tiles = []
    r = cap
    while r > 512:
        tiles.append(512)
        r -= 512
    tiles.append(r)
    return sorted(tiles)


def build_program(caps, loop_reps=1, unroll=4):
    """caps: per-slot token capacities (multiples of 128), len == EPC."""
    import concourse.mybir as mybir
    import concourse.tile as tile
    from concourse import bacc

    f32 = mybir.dt.float32
    bf16 = mybir.dt.bfloat16

    slot_tiles = [_split_tiles(c) for c in caps]
    cap_total = sum(caps)
    # xt/yt are packed [slot0 | slot1] along the token axis
    slot_off = [sum(caps[:s]) for s in range(EPC)]

    nc = bacc.Bacc("TRN2", target_bir_lowering=False, debug=False)
    xt = nc.dram_tensor("xt", [D_MODEL, cap_total], bf16, kind="ExternalInput").ap()
    w1 = nc.dram_tensor("w1", [EPC, D_MODEL, D_HIDDEN], bf16, kind="ExternalInput").ap()
    b1 = nc.dram_tensor("b1", [EPC, D_HIDDEN], f32, kind="ExternalInput").ap()
    w2 = nc.dram_tensor("w2", [EPC, D_HIDDEN, D_MODEL], bf16, kind="ExternalInput").ap()
    b2 = nc.dram_tensor("b2", [EPC, D_MODEL], f32, kind="ExternalInput").ap()
    yt = nc.dram_tensor("yt", [D_MODEL, cap_total], bf16, kind="ExternalOutput").ap()

    Relu = mybir.ActivationFunctionType.Relu

    xt_src = xt.rearrange("(c p) n -> p c n", p=128)
    yt_dst = yt.rearrange("(c p) n -> p c n", p=128)

    with tile.TileContext(nc) as tc:
        with (
            tc.tile_pool(name="w1p", bufs=2) as w1p,
            tc.tile_pool(name="w2p", bufs=2) as w2p,
            tc.tile_pool(name="bp", bufs=2) as bp,
            tc.tile_pool(name="xp", bufs=2) as xp,
            tc.tile_pool(name="hp", bufs=2) as hp,
            tc.tile_pool(name="yp", bufs=2) as yp,
            tc.tile_pool(name="ps1", bufs=4, space="PSUM") as ps1,
            tc.tile_pool(name="ps2", bufs=4, space="PSUM") as ps2,
        ):
            if loop_reps > 1:
                assert loop_reps % unroll == 0, (loop_reps, unroll)
                loop_cm = tc.For_i(0, loop_reps // unroll, 1)
            else:
                loop_cm = contextlib.nullcontext()

            def enqueue_loads(e, NQ, split_x0):
                """Enqueue slot e's weight/bias/x DMAs; returns tile handles."""
                tiles = slot_tiles[e]
                T = len(tiles)
                off = [slot_off[e] + sum(tiles[:j]) for j in range(T)]
                w1_src = w1[e].rearrange("(c p) m -> p c m", p=128)
                w2_src = w2[e].rearrange("(c p) m -> p c m", p=128)

                # sync queue: W1 chunks + biases
                w1t = w1p.tile([128, KC1, D_HIDDEN], bf16, tag="w1t",
                               name=f"w1t_{e}")
                QW = D_HIDDEN // NQ
                nc.sync.dma_start(w1t[:, :, :QW], w1_src[:, :, :QW])
                b1t = bp.tile([128, MC1], f32, tag="b1t", name=f"b1t_{e}")
                nc.sync.dma_start(b1t[:], b1[e].rearrange("(m p) -> p m", p=128))
                b2t = bp.tile([128, MC2], f32, tag="b2t", name=f"b2t_{e}")
                nc.sync.dma_start(b2t[:], b2[e].rearrange("(m p) -> p m", p=128))
                for q in range(1, NQ):
                    nc.sync.dma_start(
                        w1t[:, :, q * QW : (q + 1) * QW],
                        w1_src[:, :, q * QW : (q + 1) * QW],
                    )

                # gpsimd queue: xt tiles, W2 chunks
                xtiles = []
                for j, nt in enumerate(tiles):
                    xtiles.append(
                        xp.tile([128, KC1, nt], bf16, tag="xtile",
                                name=f"xtile_{e}_{j}")
                    )
                if split_x0:
                    for cc in range(0, KC1, 2):
                        nc.gpsimd.dma_start(
                            xtiles[0][:, cc : cc + 2, :],
                            xt_src[:, cc : cc + 2, off[0] : off[0] + tiles[0]],
                        )
                else:
                    nc.gpsimd.dma_start(
                        xtiles[0][:], xt_src[:, :, off[0] : off[0] + tiles[0]]
                    )
                if T > 1:
                    nc.gpsimd.dma_start(
                        xtiles[1][:], xt_src[:, :, off[1] : off[1] + tiles[1]]
                    )
                w2t = w2p.tile([128, KC2, D_MODEL], bf16, tag="w2t",
                               name=f"w2t_{e}")
                QW2 = D_MODEL // NQ
                for q in range(NQ):
                    nc.gpsimd.dma_start(
                        w2t[:, :, q * QW2 : (q + 1) * QW2],
                        w2_src[:, :, q * QW2 : (q + 1) * QW2],
                    )
                for j in range(2, T):
                    nc.gpsimd.dma_start(
                        xtiles[j][:], xt_src[:, :, off[j] : off[j] + tiles[j]]
                    )
                return {"w1t": w1t, "w2t": w2t, "b1t": b1t, "b2t": b2t,
                        "xtiles": xtiles}

            def compute(e, h):
                tiles = slot_tiles[e]
                T = len(tiles)
                off = [slot_off[e] + sum(tiles[:j]) for j in range(T)]
                w1t, w2t = h["w1t"], h["w2t"]
                b1t, b2t = h["b1t"], h["b2t"]
                xtiles = h["xtiles"]
                hts = [None] * T

                def layer1(j):
                    nt = tiles[j]
                    ht = hp.tile([128, KC2, nt], bf16, tag="ht",
                                 name=f"ht_{e}_{j}")
                    hts[j] = ht
                    for m in range(MC1):
                        hps = ps1.tile([128, nt], f32, tag="hps")
                        for c in range(KC1):
                            nc.tensor.matmul(
                                hps[:],
                                lhsT=w1t[:, c, m * 128 : (m + 1) * 128],
                                rhs=xtiles[j][:, c, :],
                                start=(c == 0),
                                stop=(c == KC1 - 1),
                            )
                        nc.scalar.activation(
                            ht[:, m, :], hps[:], Relu, bias=b1t[:, m : m + 1]
                        )

                def layer2(j):
                    nt = tiles[j]
                    ht = hts[j]
                    ysb = yp.tile([128, MC2, nt], bf16, tag="ysb",
                                  name=f"ysb_{e}_{j}")
                    for m in range(MC2):
                        yps = ps2.tile([128, nt], f32, tag="yps")
                        for c in range(KC2):
                            nc.tensor.matmul(
                                yps[:],
                                lhsT=w2t[:, c, m * 128 : (m + 1) * 128],
                                rhs=ht[:, c, :],
                                start=(c == 0),
                                stop=(c == KC2 - 1),
                            )
                        nc.vector.tensor_scalar_add(
                            ysb[:, m, :], yps[:], b2t[:, m : m + 1]
                        )
                    nc.scalar.dma_start(
                        yt_dst[:, :, off[j] : off[j] + nt], ysb[:]
                    )

                for k in range(T + 1):
                    if k < T:
                        layer1(k)
                    if k >= 1:
                        layer2(k - 1)

            # U-way unroll: the For_i all-engine barrier bubble (weight-load
            # head after the barrier) is paid once per U logical iterations.
            U = unroll
            with loop_cm:
                for u in range(U if loop_reps > 1 else 1):
                    for e in range(EPC):
                        h = enqueue_loads(e, NQ=2 if (u or e) else 4,
                                          split_x0=not (u or e))
                        compute(e, h)
    nc.compile()
    return nc
def route(x, Wg):
    logits = x.astype(np.float32, copy=False) @ Wg.astype(np.float32, copy=False).T
    n = logits.shape[0]
    rows = np.arange(n)
    i1 = np.argmax(logits, axis=1)
    v1 = logits[rows, i1]
    masked = logits.copy()
    masked[rows, i1] = -np.inf
    i2 = np.argmax(masked, axis=1)
    v2 = masked[rows, i2]
    d = np.exp((v2 - v1).astype(np.float64))
    wt1 = (1.0 / (1.0 + d)).astype(np.float32)
    wt2 = (d / (1.0 + d)).astype(np.float32)
    return i1, i2, wt1, wt2


def plan(idxs):
    """Assign experts to (core, slot): largest 8 -> slot 0, smallest 8 ->
    slot 1; biggest big pairs with smallest small. Returns (assign[core] ->
    [expert_slot0, expert_slot1], caps per slot)."""
    order = sorted(range(N_EXPERTS), key=lambda e: -len(idxs[e]))
    big, small = order[:N_CORES], order[N_CORES:]
    small = small[::-1]
    assign = [[big[c], small[c]] for c in range(N_CORES)]
    caps = []
    for s in range(EPC):
        mx = max(len(idxs[assign[c][s]]) for c in range(N_CORES))
        caps.append(max(128, -(-mx // 128) * 128))
    return assign, caps


def make_in_maps(x, W1, b1, W2, b2, idxs, assign, caps):
    import ml_dtypes

    bf = ml_dtypes.bfloat16
    cap_total = sum(caps)
    slot_off = [sum(caps[:s]) for s in range(EPC)]
    in_maps = []
    for core in range(N_CORES):
        xt = np.zeros((D_MODEL, cap_total), dtype=bf)
        es = assign[core]
        for s in range(EPC):
            e = es[s]
            o = slot_off[s]
            xt[:, o : o + len(idxs[e])] = x[idxs[e]].T.astype(bf)
        in_maps.append(
            {
                "xt": xt,
                "w1": np.ascontiguousarray(W1[es]).astype(bf),
                "b1": np.ascontiguousarray(b1[es]),
                "w2": np.ascontiguousarray(W2[es]).astype(bf),
                "b2": np.ascontiguousarray(b2[es]),
            }
        )
    return in_maps


def kernel(x, Wg, W1, b1, W2, b2):
    from concourse.bass_utils import run_bass_kernel_spmd

    x = np.ascontiguousarray(np.asarray(x, dtype=np.float32))
    Wg = np.asarray(Wg, dtype=np.float32)
    W1 = np.asarray(W1, dtype=np.float32)
    b1 = np.asarray(b1, dtype=np.float32)
    W2 = np.asarray(W2, dtype=np.float32)
    b2 = np.asarray(b2, dtype=np.float32)
    n_tokens = x.shape[0]

    i1, i2, wt1, wt2 = route(x, Wg)

    idxs, wts = [], []
    for e in range(N_EXPERTS):
        sel1 = i1 == e
        sel2 = i2 == e
        idxs.append(np.concatenate([np.nonzero(sel1)[0], np.nonzero(sel2)[0]]))
        wts.append(np.concatenate([wt1[sel1], wt2[sel2]]))

    assign, caps = plan(idxs)
    in_maps = make_in_maps(x, W1, b1, W2, b2, idxs, assign, caps)
    nc = build_program(caps)
    res = run_bass_kernel_spmd(nc, in_maps, core_ids=list(range(N_CORES)))

    slot_off = [sum(caps[:s]) for s in range(EPC)]
    out = np.zeros((n_tokens, D_MODEL), dtype=np.float32)
    for core in range(N_CORES):
        for s in range(EPC):
            e = assign[core][s]
            n_e = len(idxs[e])
            if n_e == 0:
                continue
            o = slot_off[s]
            y = res.results[core]["yt"][:, o : o + n_e].T.astype(np.float32)
            out[idxs[e]] += wts[e][:, None] * y
    return out
